# revision 35
# baseline (speedup 1.0000x reference)
"""Trainium2 Bass kernel for nn_Encoder_WordLstm (bi-LSTM over char/bichar embeddings).

Sharding: data-parallel over batch. Each of the 8 cores handles 8 sentences and
runs BOTH LSTM directions as two decoupled dependency chains that interleave on
the engines. Matmul operands are bf16 (fp32 PSUM accumulate); embedding tables
are pre-cast to bf16 (halves gather DMA traffic and makes PE transposes
1 cycle/row).

Per-core pipeline (all on device):
  1. indirect-DMA gathers: char/schar streams shared by both sides; bichar
     per side -> feat [128tok, 400+400] bf16 (all in forward token order)
  2. PE transpose -> featT, matmul W_lin + tanh -> linT [300, 128tok] bf16
  3. matmul Wih (bias via ones-row augmentation) -> x tiles (bf16) -> DRAM
  4. 512-step LSTM recurrence, both chains emitted op-interleaved:
     - gates = x + h @ WhhT via h-stationary matmuls (hT tiles as lhsT, Whh
       streamed); x and Whh-tail rows ride the k3 matmul via an I8-augmented
       stationary; the right chain reads x at reversed step indices.
     - ACT: sigmoid(i,f) fused, tanh(g), sigmoid(o); DVE: A=i*g~, D=f*c,
       c'=A+D.
     - transposed tail: PE-transpose o and c' into PSUM [128,24], tanh(c'T)
       and hT = oT*tanh(c'T) written straight into the double-buffered hT
       state tiles (no per-step copies); hs output DMA'd transposed.
Host reassembles/unpermutes to [64, 512, 600] f32.
"""

import os
import sys

import numpy as np

sys.path.insert(0, "/opt/trn_rl_repo")

import concourse.bass as bass
import concourse.bacc as bacc
import concourse.mybir as mybir
import concourse.tile as tile
from concourse.bass_utils import run_bass_kernel_spmd
from concourse.masks import make_identity

F32 = mybir.dt.float32
BF16 = mybir.dt.bfloat16
I32 = mybir.dt.int32
AF = mybir.ActivationFunctionType
ALU = mybir.AluOpType

B_TOT, S = 64, 512
DC = DB = 200
HID = H = 300
VC, VB = 10000, 200000
NCORES = 8
BL = B_TOT // NCORES          # 8 sentences per core
T = BL * S                    # 4096 tokens per core
G4 = 4 * H                    # 1200

# smoke-test overrides (break numerics, only to exercise compile/run quickly)
N_TILES = int(os.environ.get("K_NTILES", T // 128))   # 32
STEPS = int(os.environ.get("K_STEPS", S))             # 512
ABL = os.environ.get("K_ABL", "")          # ablation flags: hs,xdma,tp

# gate permutation: torch/ref order (i,f,g,o) -> kernel order (i,f,o,g)
_PERM = np.r_[0:300, 300:600, 900:1200, 600:900]

M300 = [128, 128, 44]         # chunks of 300 (lin output dims / recurrence h)
N512 = [(0, 512), (512, 512), (1024, 176)]  # free-dim chunks of 1200
KXP = [128, 128, 65]          # xproj contraction chunks (65 = 44 dims + ones@64)
RB = 32                       # right chain's partition base (32-aligned)


def _build_program():
    nc = bacc.Bacc()

    idx_d = nc.declare_dram_parameter("idx", [128, N_TILES * 6], I32, isOutput=False)
    tab_char = nc.declare_dram_parameter("char_embed", [VC, DC], BF16, isOutput=False)
    tab_schar = nc.declare_dram_parameter("static_char_embed", [VC, DC], BF16, isOutput=False)
    tab_bi = nc.declare_dram_parameter("bichar_embed", [VB, DB], BF16, isOutput=False)
    tab_sbi = nc.declare_dram_parameter("static_bichar_embed", [VB, DB], BF16, isOutput=False)
    wlin_d = nc.declare_dram_parameter("wlin_blk", [128, 24 * 128], BF16, isOutput=False)
    blin_d = nc.declare_dram_parameter("blin_blk", [128, 3], F32, isOutput=False)
    wih_d = nc.declare_dram_parameter("wihaug_blk", [128, 2 * 3 * G4], BF16, isOutput=False)
    whh12_d = nc.declare_dram_parameter("whh12_blk", [128, 2 * 2 * G4], BF16, isOutput=False)
    whh3_d = nc.declare_dram_parameter("whh3_blk", [44, 2 * G4], BF16, isOutput=False)
    i8_d = nc.declare_dram_parameter("i8blk", [8, 8], BF16, isOutput=False)
    ones_d = nc.declare_dram_parameter("onesblk", [1, 128], BF16, isOutput=False)
    hs12_d = nc.declare_dram_parameter("hs12", [2, S, 128, 16], BF16, isOutput=True)
    hs3_d = nc.declare_dram_parameter("hs3", [2, S, 44, 8], BF16, isOutput=True)
    x_d = nc.dram_tensor("x_seq", [2, T, G4], BF16)

    tables = [tab_char, tab_schar, tab_bi, tab_sbi]

    with tile.TileContext(nc) as tc:
        with (
            tc.tile_pool(name="const", bufs=1) as cp,
            tc.tile_pool(name="ph_sb", bufs=2) as pp,
            tc.tile_pool(name="rc_sb", bufs=2) as rp,
            tc.tile_pool(name="rc_h", bufs=4) as hp,
            tc.tile_pool(name="ps", bufs=1, space="PSUM") as psp,
        ):
            ident = cp.tile([128, 128], F32, tag="ident")
            make_identity(nc, ident[:, :])
            identb = cp.tile([128, 128], BF16, tag="identb")
            make_identity(nc, identb[:, :])
            idx_sb = cp.tile([128, N_TILES * 6], I32, tag="idx")
            nc.sync.dma_start(out=idx_sb[:, :], in_=idx_d[:, :])
            wlin_sb = cp.tile([128, 24 * 128], BF16, tag="wlin")
            nc.sync.dma_start(out=wlin_sb[:, :], in_=wlin_d[:, :])
            blin_sb = cp.tile([128, 3], F32, tag="blin")
            nc.sync.dma_start(out=blin_sb[:, :], in_=blin_d[:, :])
            wih_sb = cp.tile([128, 2 * 3 * G4], BF16, tag="wih")
            nc.sync.dma_start(out=wih_sb[:, :], in_=wih_d[:, :])
            whh12_sb = cp.tile([128, 2 * 2 * G4], BF16, tag="whh12")
            nc.sync.dma_start(out=whh12_sb[:, :], in_=whh12_d[:, :])

            # persistent linT tiles (side x parity); ones row 64 loaded once
            linTs = {}
            for side in range(2):
                for par in range(2):
                    lt = cp.tile([128, 3 * 128], BF16, tag=f"linT_{side}_{par}")
                    nc.sync.dma_start(out=lt[64:65, 256:384], in_=ones_d[:, :])
                    linTs[(side, par)] = lt

            # recurrence state, fully per-chain to keep the two scans decoupled.
            # hT state double-buffered (t%2) so the per-step hs-output DMA read
            # doesn't stall the next step's write (WAR slack of 2 steps).
            hT12s, hT3s, c_sts, b3s = [], [], [], []
            for c in range(2):
                p12, p3 = [], []
                for r in range(2):
                    t12 = cp.tile([128, 16], BF16, tag=f"hT12_{c}_{r}")
                    nc.vector.memset(t12[:, :], 0.0)
                    t3 = cp.tile([52, 8], BF16, tag=f"hT3_{c}_{r}")
                    nc.vector.memset(t3[0:44, :], 0.0)
                    nc.sync.dma_start(out=t3[44:52, 0:8], in_=i8_d[:, :])
                    p12.append(t12); p3.append(t3)
                cs = cp.tile([8, H], F32, tag=f"c_{c}")
                nc.vector.memset(cs[:, :], 0.0)
                hT12s.append(p12); hT3s.append(p3); c_sts.append(cs)
                bufs = []
                for r in range(4):
                    b3 = cp.tile([52, G4], BF16, tag=f"b3_{c}_{r}")
                    nc.sync.dma_start(out=b3[0:44, :], in_=whh3_d[0:44, c * G4:(c + 1) * G4])
                    bufs.append(b3)
                b3s.append(bufs)

            # ---------------- phases 1-3: gather, transpose, linear, xproj ----
            def emit_tile(t):
                # char/schar gathers shared by both sides (right side consumes
                # x in reversed step order instead of gathering reversed)
                featcs = pp.tile([128, 400], BF16, tag="featcs")
                for j2 in range(2):
                    nc.gpsimd.indirect_dma_start(
                        out=featcs[:, 200 * j2:200 * (j2 + 1)],
                        out_offset=None,
                        in_=tables[j2][:, :],
                        in_offset=bass.IndirectOffsetOnAxis(
                            ap=idx_sb[:, t * 6 + j2:t * 6 + j2 + 1], axis=0),
                    )
                featTcs = pp.tile([128, 4 * 128], BF16, tag="ftcs")
                for kc in range(4):
                    tp = psp.tile([128, 128], BF16, tag="plg1", bufs=2)
                    nc.tensor.transpose(
                        tp[0:100, 0:128], featcs[:, kc * 100:(kc + 1) * 100],
                        identb[:, :])
                    nc.vector.tensor_copy(
                        featTcs[0:100, kc * 128:(kc + 1) * 128], tp[0:100, 0:128])
                for side in range(2):
                    featb = pp.tile([128, 400], BF16, tag=f"featb{side}")
                    for j2 in range(2):
                        col = t * 6 + 2 + side * 2 + j2
                        nc.gpsimd.indirect_dma_start(
                            out=featb[:, 200 * j2:200 * (j2 + 1)],
                            out_offset=None,
                            in_=tables[2 + j2][:, :],
                            in_offset=bass.IndirectOffsetOnAxis(
                                ap=idx_sb[:, col:col + 1], axis=0),
                        )
                    featTb = pp.tile([128, 4 * 128], BF16, tag=f"ftb{side}")
                    for kc in range(4):
                        tp = psp.tile([128, 128], BF16, tag="plg1", bufs=2)
                        nc.tensor.transpose(
                            tp[0:100, 0:128], featb[:, kc * 100:(kc + 1) * 100],
                            identb[:, :])
                        nc.vector.tensor_copy(
                            featTb[0:100, kc * 128:(kc + 1) * 128], tp[0:100, 0:128])
                    linT = linTs[(side, t % 2)]
                    for m in range(3):
                        mm = M300[m]
                        pl = psp.tile([128, 128], F32, tag="plg1", bufs=2)
                        for kc in range(8):
                            blk = (kc * 3 + m) * 128
                            rhsT = (featTcs if kc < 4 else featTb)
                            rkc = kc if kc < 4 else kc - 4
                            nc.tensor.matmul(
                                pl[0:mm, 0:128],
                                lhsT=wlin_sb[0:100, blk:blk + mm],
                                rhs=rhsT[0:100, rkc * 128:(rkc + 1) * 128],
                                start=(kc == 0), stop=(kc == 7))
                        nc.scalar.activation(
                            linT[0:mm, m * 128:m * 128 + 128],
                            pl[0:mm, 0:128], AF.Tanh,
                            bias=blin_sb[0:mm, m:m + 1])
                    px = psp.tile([128, G4], F32, tag="pxg0", bufs=2)
                    for kc in range(3):
                        kw = KXP[kc]
                        for (n0, nw) in N512:
                            nc.tensor.matmul(
                                px[:, n0:n0 + nw],
                                lhsT=linT[0:kw, kc * 128:kc * 128 + 128],
                                rhs=wih_sb[0:kw, (side * 3 + kc) * G4 + n0:
                                           (side * 3 + kc) * G4 + n0 + nw],
                                start=(kc == 0), stop=(kc == 2))
                    x_sb = pp.tile([128, G4], BF16, tag=f"x{side}")
                    for (n0, nw) in N512:
                        nc.scalar.copy(x_sb[:, n0:n0 + nw], px[:, n0:n0 + nw])
                    nc.sync.dma_start(
                        out=x_d[side, t * 128:(t + 1) * 128, :], in_=x_sb[:, :])

            # ---------------- phase 4: the two LSTM scans ---------------------
            # Per-chain dependency chains (independent, interleaved on engines).
            # c' = f*c + 2*(i*s) - i  where s = sigmoid(2g)  [tanh-free g path]
            # Op-level interleaving of the two chains: the ACT/DVE/Pool engine
            # queues are strict FIFO (exec-queue depth 0/8), so emitting chain
            # L's full step then chain R's causes head-of-line blocking (R's
            # ready sigmoid queued behind L's not-yet-ready tanh_c). Emitting
            # op-by-op across chains lets each engine alternate chains.
            abl = set(ABL.split(","))
            idn = identb[0:8, 0:8]
            idnf = ident[0:8, 0:8]

            def emit_step(t):
                ps_t, sg_t, so_t, tpx_t, tcT_t = {}, {}, {}, {}, {}
                rd, wr = (t + 1) % 2, t % 2
                for c in range(2):
                    b3 = b3s[c][t % 4]
                    tx = t if c == 0 else (S - 1 - t)
                    if "xdma" not in abl:
                        nc.sync.dma_start(
                            out=b3[44:52, :], in_=x_d[c, tx * 8:(tx + 1) * 8, :])
                    ps = psp.tile([8, G4], F32, tag="pxg0", bufs=2)
                    ps_t[c] = ps
                    for (n0, nw) in N512:
                        nc.tensor.matmul(
                            ps[:, n0:n0 + nw],
                            lhsT=hT12s[c][rd][:, 0:8],
                            rhs=whh12_sb[:, (c * 2) * G4 + n0:(c * 2) * G4 + n0 + nw],
                            start=True, stop=False)
                        nc.tensor.matmul(
                            ps[:, n0:n0 + nw],
                            lhsT=hT12s[c][rd][:, 8:16],
                            rhs=whh12_sb[:, (c * 2 + 1) * G4 + n0:
                                         (c * 2 + 1) * G4 + n0 + nw],
                            start=False, stop=False)
                        nc.tensor.matmul(
                            ps[:, n0:n0 + nw],
                            lhsT=hT3s[c][rd][0:52, 0:8],
                            rhs=b3[0:52, n0:n0 + nw], start=False, stop=True)
                # gate cols after PERM: i 0:300, f 300:600, o 600:900, g 900:1200
                for c in range(2):
                    sg = rp.tile([8, 600], BF16, tag=f"sg{c}")
                    sg_t[c] = sg
                    nc.scalar.activation(sg[:, 0:600], ps_t[c][:, 0:600], AF.Sigmoid)
                for c in range(2):
                    sgg = rp.tile([8, H], BF16, tag=f"sgg{c}")
                    nc.scalar.activation(sgg[:, :], ps_t[c][:, 900:1200], AF.Tanh)
                    sg_t[c] = (sg_t[c], sgg)
                for c in range(2):
                    so = rp.tile([8, H], F32, tag=f"so{c}")
                    so_t[c] = so
                    nc.scalar.activation(so[:, :], ps_t[c][:, 600:900], AF.Sigmoid)
                Dts = {}
                for c in range(2):
                    Dt = rp.tile([8, H], F32, tag=f"D{c}")
                    Dts[c] = Dt
                    nc.vector.tensor_tensor(
                        Dt[:, :], sg_t[c][0][:, 300:600], c_sts[c][:, :],
                        op=ALU.mult)
                Aps = {}
                for c in range(2):
                    Ap = rp.tile([8, H], BF16, tag=f"A{c}")
                    Aps[c] = Ap
                    nc.vector.tensor_tensor(
                        Ap[:, :], sg_t[c][0][:, 0:300], sg_t[c][1][:, :],
                        op=ALU.mult)
                for c in range(2):
                    nc.vector.tensor_tensor(
                        c_sts[c][:, :], Aps[c][:, :], Dts[c][:, :], op=ALU.add)
                # transposed tail: transpose o and c', tanh + h-mult in
                # [128,24] space straight into the hT state tiles (no copies)
                for c in range(2):
                    tpx = psp.tile([128, 48], F32, tag="plg1", bufs=2)
                    tpx_t[c] = tpx
                    nc.tensor.transpose(tpx[:, 0:8], so_t[c][:, 0:128], idnf)
                    nc.tensor.transpose(tpx[:, 8:16], so_t[c][:, 128:256], idnf)
                    nc.tensor.transpose(tpx[0:44, 16:24], so_t[c][:, 256:300], idnf)
                for c in range(2):
                    tpx = tpx_t[c]
                    nc.tensor.transpose(tpx[:, 24:32], c_sts[c][:, 0:128], idnf)
                    nc.tensor.transpose(tpx[:, 32:40], c_sts[c][:, 128:256], idnf)
                    nc.tensor.transpose(tpx[0:44, 40:48], c_sts[c][:, 256:300], idnf)
                for c in range(2):
                    tcT = rp.tile([128, 24], BF16, tag=f"tcT{c}")
                    tcT_t[c] = tcT
                    nc.scalar.activation(tcT[:, 0:16], tpx_t[c][:, 24:40], AF.Tanh)
                    nc.scalar.activation(
                        tcT[0:44, 16:24], tpx_t[c][0:44, 40:48], AF.Tanh)
                for c in range(2):
                    nc.vector.tensor_tensor(
                        hT12s[c][wr][:, 0:16], tpx_t[c][:, 0:16],
                        tcT_t[c][:, 0:16], op=ALU.mult)
                    nc.vector.tensor_tensor(
                        hT3s[c][wr][0:44, 0:8], tpx_t[c][0:44, 16:24],
                        tcT_t[c][0:44, 16:24], op=ALU.mult)
                if "hs" not in abl:
                    for c in range(2):
                        nc.sync.dma_start(
                            out=hs12_d[c, t, :, :], in_=hT12s[c][wr][:, 0:16])
                        nc.sync.dma_start(
                            out=hs3_d[c, t, :, :], in_=hT3s[c][wr][0:44, 0:8])

            for k in range(N_TILES):
                emit_tile(k)
            for t in range(STEPS):
                emit_step(t)
    nc.compile()
    return nc


def _prep_host(inputs):
    """Build the per-core in_maps (host-side weight/index preprocessing)."""
    f = {k: np.asarray(v) for k, v in inputs.items()}

    wlinT = f["W_lin"].astype(np.float32).T            # [800, 300]
    wlin_blk = np.zeros((128, 24 * 128), np.float32)
    for kc in range(8):
        for m in range(3):
            mm = M300[m]
            blk = (kc * 3 + m) * 128
            wlin_blk[0:100, blk:blk + mm] = wlinT[kc * 100:(kc + 1) * 100,
                                                 m * 128:m * 128 + mm]
    blin_blk = np.zeros((128, 3), np.float32)
    for m in range(3):
        mm = M300[m]
        blin_blk[0:mm, m] = f["b_lin"][m * 128:m * 128 + mm]

    wih_blk = np.zeros((128, 2 * 3 * G4), np.float32)
    whh12_blk = np.zeros((128, 2 * 2 * G4), np.float32)
    whh3_blk = np.zeros((44, 2 * G4), np.float32)
    gscale = np.ones((G4,), np.float32)   # direct tanh on g: no prescale
    for c, sfx in enumerate(("l", "r")):
        wihT = (f[f"Wih_{sfx}"][_PERM, :].astype(np.float32) * gscale[:, None]).T
        bb = f[f"b_{sfx}"][_PERM].astype(np.float32) * gscale
        for kc in range(2):
            wih_blk[0:128, (c * 3 + kc) * G4:(c * 3 + kc + 1) * G4] = \
                wihT[kc * 128:(kc + 1) * 128, :]
        wih_blk[0:44, (c * 3 + 2) * G4:(c * 3 + 3) * G4] = wihT[256:300, :]
        wih_blk[64, (c * 3 + 2) * G4:(c * 3 + 3) * G4] = bb
        whhT = (f[f"Whh_{sfx}"][_PERM, :].astype(np.float32) * gscale[:, None]).T
        whh12_blk[:, (c * 2) * G4:(c * 2 + 1) * G4] = whhT[0:128, :]
        whh12_blk[:, (c * 2 + 1) * G4:(c * 2 + 2) * G4] = whhT[128:256, :]
        whh3_blk[:, c * G4:(c + 1) * G4] = whhT[256:300, :]

    import ml_dtypes
    bf = lambda a: a.astype(ml_dtypes.bfloat16)
    shared = {
        "char_embed": bf(f["char_embed"]),
        "static_char_embed": bf(f["static_char_embed"]),
        "bichar_embed": bf(f["bichar_embed"]),
        "static_bichar_embed": bf(f["static_bichar_embed"]),
        "wlin_blk": bf(wlin_blk), "blin_blk": blin_blk,
        "wihaug_blk": bf(wih_blk), "whh12_blk": bf(whh12_blk),
        "whh3_blk": bf(whh3_blk),
        "i8blk": bf(np.eye(8, dtype=np.float32)),
        "onesblk": bf(np.ones((1, 128), np.float32)),
    }

    in_maps = []
    for core in range(NCORES):
        bs = slice(core * BL, (core + 1) * BL)
        idx_blk = np.zeros((128, N_TILES * 6), np.int32)
        # stream order: [char schar bl sbl br sbr], all forward token order
        streams = [
            f["char_features"][bs].T.reshape(-1),
            f["static_char_features"][bs].T.reshape(-1),
            f["bichar_left_features"][bs].T.reshape(-1),
            f["static_bichar_left_features"][bs].T.reshape(-1),
            f["bichar_right_features"][bs].T.reshape(-1),
            f["static_bichar_right_features"][bs].T.reshape(-1),
        ]
        for t in range(N_TILES):
            for j in range(6):
                idx_blk[:, t * 6 + j] = streams[j][t * 128:(t + 1) * 128]
        in_maps.append({"idx": idx_blk, **shared})
    return in_maps


_CACHED = {}


def kernel(**inputs):
    if "nc" not in _CACHED:
        _CACHED["nc"] = _build_program()
    nc = _CACHED["nc"]
    in_maps = _prep_host(inputs)
    res = run_bass_kernel_spmd(nc, in_maps, list(range(NCORES)))
    _CACHED["last_result"] = res
    out = np.empty((B_TOT, S, 2 * H), np.float32)
    for core in range(NCORES):
        # hs12 [2,S,128p,16=(2k,8b)] -> h[c,t,b,128k+p]; hs3 [2,S,44p,8b]
        h12 = res.results[core]["hs12"].astype(np.float32)
        h3 = res.results[core]["hs3"].astype(np.float32)
        hs = np.empty((2, S, BL, H), np.float32)
        hs[:, :, :, 0:256] = h12.reshape(2, S, 128, 2, 8).transpose(0, 1, 4, 3, 2).reshape(2, S, 8, 256)
        hs[:, :, :, 256:300] = h3.transpose(0, 1, 3, 2)
        bs = slice(core * BL, (core + 1) * BL)
        out[bs, :, 0:H] = hs[0].transpose(1, 0, 2)
        out[bs, :, H:2 * H] = hs[1, ::-1].transpose(1, 0, 2)
    return out


if __name__ == "__main__":
    sys.path.insert(0, os.path.dirname(os.path.abspath(__file__)))
    import reference
    inp = reference.setup_inputs()
    got = kernel(**{k: np.asarray(v) for k, v in inp.items()})
    exp = np.asarray(reference.reference(**inp))
    err = np.abs(got - exp)
    rel = err.max() / np.abs(exp).max()
    print("Relative error:", rel)



# revision 36
# speedup vs baseline: 1.0112x; 1.0112x over previous
"""Trainium2 Bass kernel for nn_Encoder_WordLstm (bi-LSTM over char/bichar embeddings).

Sharding: data-parallel over batch. Each of the 8 cores handles 8 sentences and
runs BOTH LSTM directions as two decoupled dependency chains that interleave on
the engines. Matmul operands are bf16 (fp32 PSUM accumulate); embedding tables
are pre-cast to bf16 (halves gather DMA traffic and makes PE transposes
1 cycle/row).

Per-core pipeline (all on device):
  1. indirect-DMA gathers: char/schar streams shared by both sides; bichar
     per side -> feat [128tok, 400+400] bf16 (all in forward token order)
  2. PE transpose -> featT, matmul W_lin + tanh -> linT [300, 128tok] bf16
  3. matmul Wih (bias via ones-row augmentation) -> x tiles (bf16) -> DRAM
  4. 512-step LSTM recurrence, both chains emitted op-interleaved:
     - gates = x + h @ WhhT via h-stationary matmuls (hT tiles as lhsT, Whh
       streamed); x and Whh-tail rows ride the k3 matmul via an I8-augmented
       stationary; the right chain reads x at reversed step indices.
     - ACT: sigmoid(i,f) fused, tanh(g), sigmoid(o); DVE: A=i*g~, D=f*c,
       c'=A+D.
     - transposed tail: PE-transpose o and c' into PSUM [128,24], tanh(c'T)
       and hT = oT*tanh(c'T) written straight into the double-buffered hT
       state tiles (no per-step copies); hs output DMA'd transposed.
Host reassembles/unpermutes to [64, 512, 600] f32.
"""

import os
import sys

import numpy as np

sys.path.insert(0, "/opt/trn_rl_repo")

import concourse.bass as bass
import concourse.bacc as bacc
import concourse.mybir as mybir
import concourse.tile as tile
from concourse.bass_utils import run_bass_kernel_spmd
from concourse.masks import make_identity

F32 = mybir.dt.float32
BF16 = mybir.dt.bfloat16
I32 = mybir.dt.int32
AF = mybir.ActivationFunctionType
ALU = mybir.AluOpType

B_TOT, S = 64, 512
DC = DB = 200
HID = H = 300
VC, VB = 10000, 200000
NCORES = 8
BL = B_TOT // NCORES          # 8 sentences per core
T = BL * S                    # 4096 tokens per core
G4 = 4 * H                    # 1200

# smoke-test overrides (break numerics, only to exercise compile/run quickly)
N_TILES = int(os.environ.get("K_NTILES", T // 128))   # 32
STEPS = int(os.environ.get("K_STEPS", S))             # 512
ABL = os.environ.get("K_ABL", "")          # ablation flags: hs,xdma,tp

# gate permutation: torch/ref order (i,f,g,o) -> kernel order (i,f,o,g)
_PERM = np.r_[0:300, 300:600, 900:1200, 600:900]

M300 = [128, 128, 44]         # chunks of 300 (lin output dims / recurrence h)
N512 = [(0, 512), (512, 512), (1024, 176)]  # free-dim chunks of 1200
KXP = [128, 128, 65]          # xproj contraction chunks (65 = 44 dims + ones@64)
RB = 32                       # right chain's partition base (32-aligned)


def _build_program():
    nc = bacc.Bacc()

    idx_d = nc.declare_dram_parameter("idx", [128, N_TILES * 6], I32, isOutput=False)
    tab_char = nc.declare_dram_parameter("char_embed", [VC, DC], BF16, isOutput=False)
    tab_schar = nc.declare_dram_parameter("static_char_embed", [VC, DC], BF16, isOutput=False)
    tab_bi = nc.declare_dram_parameter("bichar_embed", [VB, DB], BF16, isOutput=False)
    tab_sbi = nc.declare_dram_parameter("static_bichar_embed", [VB, DB], BF16, isOutput=False)
    wlin_d = nc.declare_dram_parameter("wlin_blk", [128, 24 * 128], BF16, isOutput=False)
    blin_d = nc.declare_dram_parameter("blin_blk", [128, 3], F32, isOutput=False)
    wih_d = nc.declare_dram_parameter("wihaug_blk", [128, 2 * 3 * G4], BF16, isOutput=False)
    whh12_d = nc.declare_dram_parameter("whh12_blk", [128, 2 * 2 * G4], BF16, isOutput=False)
    whh3_d = nc.declare_dram_parameter("whh3_blk", [44, 2 * G4], BF16, isOutput=False)
    i8_d = nc.declare_dram_parameter("i8blk", [8, 8], BF16, isOutput=False)
    ones_d = nc.declare_dram_parameter("onesblk", [1, 128], BF16, isOutput=False)
    hs12_d = nc.declare_dram_parameter("hs12", [2, S, 128, 16], BF16, isOutput=True)
    hs3_d = nc.declare_dram_parameter("hs3", [2, S, 44, 8], BF16, isOutput=True)
    x_d = nc.dram_tensor("x_seq", [2, T, G4], BF16)

    tables = [tab_char, tab_schar, tab_bi, tab_sbi]

    with tile.TileContext(nc) as tc:
        with (
            tc.tile_pool(name="const", bufs=1) as cp,
            tc.tile_pool(name="ph_sb", bufs=2) as pp,
            tc.tile_pool(name="rc_sb", bufs=2) as rp,
            tc.tile_pool(name="rc_h", bufs=4) as hp,
            tc.tile_pool(name="ps", bufs=1, space="PSUM") as psp,
        ):
            ident = cp.tile([128, 128], F32, tag="ident")
            make_identity(nc, ident[:, :])
            identb = cp.tile([128, 128], BF16, tag="identb")
            make_identity(nc, identb[:, :])
            idx_sb = cp.tile([128, N_TILES * 6], I32, tag="idx")
            nc.sync.dma_start(out=idx_sb[:, :], in_=idx_d[:, :])
            wlin_sb = cp.tile([128, 24 * 128], BF16, tag="wlin")
            nc.sync.dma_start(out=wlin_sb[:, :], in_=wlin_d[:, :])
            blin_sb = cp.tile([128, 3], F32, tag="blin")
            nc.sync.dma_start(out=blin_sb[:, :], in_=blin_d[:, :])
            wih_sb = cp.tile([128, 2 * 3 * G4], BF16, tag="wih")
            nc.sync.dma_start(out=wih_sb[:, :], in_=wih_d[:, :])
            whh12_sb = cp.tile([128, 2 * 2 * G4], BF16, tag="whh12")
            nc.sync.dma_start(out=whh12_sb[:, :], in_=whh12_d[:, :])

            # persistent linT tiles (side x parity); ones row 64 loaded once
            linTs = {}
            for side in range(2):
                for par in range(2):
                    lt = cp.tile([128, 3 * 128], BF16, tag=f"linT_{side}_{par}")
                    nc.sync.dma_start(out=lt[64:65, 256:384], in_=ones_d[:, :])
                    linTs[(side, par)] = lt

            # recurrence state, fully per-chain to keep the two scans decoupled.
            # hT state double-buffered (t%2) so the per-step hs-output DMA read
            # doesn't stall the next step's write (WAR slack of 2 steps).
            hT12s, hT3s, c_sts, b3s = [], [], [], []
            for c in range(2):
                p12, p3 = [], []
                for r in range(2):
                    t12 = cp.tile([128, 16], BF16, tag=f"hT12_{c}_{r}")
                    nc.vector.memset(t12[:, :], 0.0)
                    t3 = cp.tile([52, 8], BF16, tag=f"hT3_{c}_{r}")
                    nc.vector.memset(t3[0:44, :], 0.0)
                    nc.sync.dma_start(out=t3[44:52, 0:8], in_=i8_d[:, :])
                    p12.append(t12); p3.append(t3)
                cs = cp.tile([8, H], BF16, tag=f"c_{c}")
                nc.vector.memset(cs[:, :], 0.0)
                hT12s.append(p12); hT3s.append(p3); c_sts.append(cs)
                bufs = []
                for r in range(4):
                    b3 = cp.tile([52, G4], BF16, tag=f"b3_{c}_{r}")
                    nc.sync.dma_start(out=b3[0:44, :], in_=whh3_d[0:44, c * G4:(c + 1) * G4])
                    bufs.append(b3)
                b3s.append(bufs)

            # ---------------- phases 1-3: gather, transpose, linear, xproj ----
            def emit_tile(t):
                # char/schar gathers shared by both sides (right side consumes
                # x in reversed step order instead of gathering reversed)
                featcs = pp.tile([128, 400], BF16, tag="featcs")
                for j2 in range(2):
                    nc.gpsimd.indirect_dma_start(
                        out=featcs[:, 200 * j2:200 * (j2 + 1)],
                        out_offset=None,
                        in_=tables[j2][:, :],
                        in_offset=bass.IndirectOffsetOnAxis(
                            ap=idx_sb[:, t * 6 + j2:t * 6 + j2 + 1], axis=0),
                    )
                featTcs = pp.tile([128, 4 * 128], BF16, tag="ftcs")
                for kc in range(4):
                    tp = psp.tile([128, 128], BF16, tag="plg1", bufs=2)
                    nc.tensor.transpose(
                        tp[0:100, 0:128], featcs[:, kc * 100:(kc + 1) * 100],
                        identb[:, :])
                    nc.vector.tensor_copy(
                        featTcs[0:100, kc * 128:(kc + 1) * 128], tp[0:100, 0:128])
                for side in range(2):
                    featb = pp.tile([128, 400], BF16, tag=f"featb{side}")
                    for j2 in range(2):
                        col = t * 6 + 2 + side * 2 + j2
                        nc.gpsimd.indirect_dma_start(
                            out=featb[:, 200 * j2:200 * (j2 + 1)],
                            out_offset=None,
                            in_=tables[2 + j2][:, :],
                            in_offset=bass.IndirectOffsetOnAxis(
                                ap=idx_sb[:, col:col + 1], axis=0),
                        )
                    featTb = pp.tile([128, 4 * 128], BF16, tag=f"ftb{side}")
                    for kc in range(4):
                        tp = psp.tile([128, 128], BF16, tag="plg1", bufs=2)
                        nc.tensor.transpose(
                            tp[0:100, 0:128], featb[:, kc * 100:(kc + 1) * 100],
                            identb[:, :])
                        nc.vector.tensor_copy(
                            featTb[0:100, kc * 128:(kc + 1) * 128], tp[0:100, 0:128])
                    linT = linTs[(side, t % 2)]
                    for m in range(3):
                        mm = M300[m]
                        pl = psp.tile([128, 128], F32, tag="plg1", bufs=2)
                        for kc in range(8):
                            blk = (kc * 3 + m) * 128
                            rhsT = (featTcs if kc < 4 else featTb)
                            rkc = kc if kc < 4 else kc - 4
                            nc.tensor.matmul(
                                pl[0:mm, 0:128],
                                lhsT=wlin_sb[0:100, blk:blk + mm],
                                rhs=rhsT[0:100, rkc * 128:(rkc + 1) * 128],
                                start=(kc == 0), stop=(kc == 7))
                        nc.scalar.activation(
                            linT[0:mm, m * 128:m * 128 + 128],
                            pl[0:mm, 0:128], AF.Tanh,
                            bias=blin_sb[0:mm, m:m + 1])
                    px = psp.tile([128, G4], F32, tag="pxg0", bufs=2)
                    for kc in range(3):
                        kw = KXP[kc]
                        for (n0, nw) in N512:
                            nc.tensor.matmul(
                                px[:, n0:n0 + nw],
                                lhsT=linT[0:kw, kc * 128:kc * 128 + 128],
                                rhs=wih_sb[0:kw, (side * 3 + kc) * G4 + n0:
                                           (side * 3 + kc) * G4 + n0 + nw],
                                start=(kc == 0), stop=(kc == 2))
                    x_sb = pp.tile([128, G4], BF16, tag=f"x{side}")
                    for (n0, nw) in N512:
                        nc.scalar.copy(x_sb[:, n0:n0 + nw], px[:, n0:n0 + nw])
                    nc.sync.dma_start(
                        out=x_d[side, t * 128:(t + 1) * 128, :], in_=x_sb[:, :])

            # ---------------- phase 4: the two LSTM scans ---------------------
            # Per-chain dependency chains (independent, interleaved on engines).
            # c' = f*c + 2*(i*s) - i  where s = sigmoid(2g)  [tanh-free g path]
            # Op-level interleaving of the two chains: the ACT/DVE/Pool engine
            # queues are strict FIFO (exec-queue depth 0/8), so emitting chain
            # L's full step then chain R's causes head-of-line blocking (R's
            # ready sigmoid queued behind L's not-yet-ready tanh_c). Emitting
            # op-by-op across chains lets each engine alternate chains.
            abl = set(ABL.split(","))
            idn = identb[0:8, 0:8]
            idnf = ident[0:8, 0:8]

            def emit_step(t):
                ps_t, sg_t, so_t, tpx_t, tcT_t = {}, {}, {}, {}, {}
                rd, wr = (t + 1) % 2, t % 2
                for c in range(2):
                    b3 = b3s[c][t % 4]
                    tx = t if c == 0 else (S - 1 - t)
                    if "xdma" not in abl:
                        nc.sync.dma_start(
                            out=b3[44:52, :], in_=x_d[c, tx * 8:(tx + 1) * 8, :])
                    ps = psp.tile([8, G4], F32, tag="pxg0", bufs=2)
                    ps_t[c] = ps
                    for (n0, nw) in N512:
                        nc.tensor.matmul(
                            ps[:, n0:n0 + nw],
                            lhsT=hT12s[c][rd][:, 0:8],
                            rhs=whh12_sb[:, (c * 2) * G4 + n0:(c * 2) * G4 + n0 + nw],
                            start=True, stop=False)
                        nc.tensor.matmul(
                            ps[:, n0:n0 + nw],
                            lhsT=hT12s[c][rd][:, 8:16],
                            rhs=whh12_sb[:, (c * 2 + 1) * G4 + n0:
                                         (c * 2 + 1) * G4 + n0 + nw],
                            start=False, stop=False)
                        nc.tensor.matmul(
                            ps[:, n0:n0 + nw],
                            lhsT=hT3s[c][rd][0:52, 0:8],
                            rhs=b3[0:52, n0:n0 + nw], start=False, stop=True)
                # gate cols after PERM: i 0:300, f 300:600, o 600:900, g 900:1200
                for c in range(2):
                    sg = rp.tile([8, 600], BF16, tag=f"sg{c}")
                    sg_t[c] = sg
                    nc.scalar.activation(sg[:, 0:600], ps_t[c][:, 0:600], AF.Sigmoid)
                for c in range(2):
                    sgg = rp.tile([8, H], BF16, tag=f"sgg{c}")
                    nc.scalar.activation(sgg[:, :], ps_t[c][:, 900:1200], AF.Tanh)
                    sg_t[c] = (sg_t[c], sgg)
                for c in range(2):
                    so = rp.tile([8, H], BF16, tag=f"so{c}")
                    so_t[c] = so
                    nc.scalar.activation(so[:, :], ps_t[c][:, 600:900], AF.Sigmoid)
                Dts = {}
                for c in range(2):
                    Dt = rp.tile([8, H], BF16, tag=f"D{c}")
                    Dts[c] = Dt
                    nc.vector.tensor_tensor(
                        Dt[:, :], sg_t[c][0][:, 300:600], c_sts[c][:, :],
                        op=ALU.mult)
                Aps = {}
                for c in range(2):
                    Ap = rp.tile([8, H], BF16, tag=f"A{c}")
                    Aps[c] = Ap
                    nc.vector.tensor_tensor(
                        Ap[:, :], sg_t[c][0][:, 0:300], sg_t[c][1][:, :],
                        op=ALU.mult)
                for c in range(2):
                    nc.vector.tensor_tensor(
                        c_sts[c][:, :], Aps[c][:, :], Dts[c][:, :], op=ALU.add)
                # transposed tail: transpose o and c', tanh + h-mult in
                # [128,24] space straight into the hT state tiles (no copies)
                for c in range(2):
                    tpx = psp.tile([128, 48], BF16, tag="plg1", bufs=2)
                    tpx_t[c] = tpx
                    nc.tensor.transpose(tpx[:, 0:8], so_t[c][:, 0:128], idn)
                    nc.tensor.transpose(tpx[:, 8:16], so_t[c][:, 128:256], idn)
                    nc.tensor.transpose(tpx[0:44, 16:24], so_t[c][:, 256:300], idn)
                for c in range(2):
                    tpx = tpx_t[c]
                    nc.tensor.transpose(tpx[:, 24:32], c_sts[c][:, 0:128], idn)
                    nc.tensor.transpose(tpx[:, 32:40], c_sts[c][:, 128:256], idn)
                    nc.tensor.transpose(tpx[0:44, 40:48], c_sts[c][:, 256:300], idn)
                for c in range(2):
                    tcT = rp.tile([128, 24], BF16, tag=f"tcT{c}")
                    tcT_t[c] = tcT
                    nc.scalar.activation(tcT[:, 0:16], tpx_t[c][:, 24:40], AF.Tanh)
                    nc.scalar.activation(
                        tcT[0:44, 16:24], tpx_t[c][0:44, 40:48], AF.Tanh)
                for c in range(2):
                    nc.vector.tensor_tensor(
                        hT12s[c][wr][:, 0:16], tpx_t[c][:, 0:16],
                        tcT_t[c][:, 0:16], op=ALU.mult)
                    nc.vector.tensor_tensor(
                        hT3s[c][wr][0:44, 0:8], tpx_t[c][0:44, 16:24],
                        tcT_t[c][0:44, 16:24], op=ALU.mult)
                if "hs" not in abl:
                    for c in range(2):
                        nc.sync.dma_start(
                            out=hs12_d[c, t, :, :], in_=hT12s[c][wr][:, 0:16])
                        nc.sync.dma_start(
                            out=hs3_d[c, t, :, :], in_=hT3s[c][wr][0:44, 0:8])

            for k in range(N_TILES):
                emit_tile(k)
            for t in range(STEPS):
                emit_step(t)
    nc.compile()
    return nc


def _prep_host(inputs):
    """Build the per-core in_maps (host-side weight/index preprocessing)."""
    f = {k: np.asarray(v) for k, v in inputs.items()}

    wlinT = f["W_lin"].astype(np.float32).T            # [800, 300]
    wlin_blk = np.zeros((128, 24 * 128), np.float32)
    for kc in range(8):
        for m in range(3):
            mm = M300[m]
            blk = (kc * 3 + m) * 128
            wlin_blk[0:100, blk:blk + mm] = wlinT[kc * 100:(kc + 1) * 100,
                                                 m * 128:m * 128 + mm]
    blin_blk = np.zeros((128, 3), np.float32)
    for m in range(3):
        mm = M300[m]
        blin_blk[0:mm, m] = f["b_lin"][m * 128:m * 128 + mm]

    wih_blk = np.zeros((128, 2 * 3 * G4), np.float32)
    whh12_blk = np.zeros((128, 2 * 2 * G4), np.float32)
    whh3_blk = np.zeros((44, 2 * G4), np.float32)
    gscale = np.ones((G4,), np.float32)   # direct tanh on g: no prescale
    for c, sfx in enumerate(("l", "r")):
        wihT = (f[f"Wih_{sfx}"][_PERM, :].astype(np.float32) * gscale[:, None]).T
        bb = f[f"b_{sfx}"][_PERM].astype(np.float32) * gscale
        for kc in range(2):
            wih_blk[0:128, (c * 3 + kc) * G4:(c * 3 + kc + 1) * G4] = \
                wihT[kc * 128:(kc + 1) * 128, :]
        wih_blk[0:44, (c * 3 + 2) * G4:(c * 3 + 3) * G4] = wihT[256:300, :]
        wih_blk[64, (c * 3 + 2) * G4:(c * 3 + 3) * G4] = bb
        whhT = (f[f"Whh_{sfx}"][_PERM, :].astype(np.float32) * gscale[:, None]).T
        whh12_blk[:, (c * 2) * G4:(c * 2 + 1) * G4] = whhT[0:128, :]
        whh12_blk[:, (c * 2 + 1) * G4:(c * 2 + 2) * G4] = whhT[128:256, :]
        whh3_blk[:, c * G4:(c + 1) * G4] = whhT[256:300, :]

    import ml_dtypes
    bf = lambda a: a.astype(ml_dtypes.bfloat16)
    shared = {
        "char_embed": bf(f["char_embed"]),
        "static_char_embed": bf(f["static_char_embed"]),
        "bichar_embed": bf(f["bichar_embed"]),
        "static_bichar_embed": bf(f["static_bichar_embed"]),
        "wlin_blk": bf(wlin_blk), "blin_blk": blin_blk,
        "wihaug_blk": bf(wih_blk), "whh12_blk": bf(whh12_blk),
        "whh3_blk": bf(whh3_blk),
        "i8blk": bf(np.eye(8, dtype=np.float32)),
        "onesblk": bf(np.ones((1, 128), np.float32)),
    }

    in_maps = []
    for core in range(NCORES):
        bs = slice(core * BL, (core + 1) * BL)
        idx_blk = np.zeros((128, N_TILES * 6), np.int32)
        # stream order: [char schar bl sbl br sbr], all forward token order
        streams = [
            f["char_features"][bs].T.reshape(-1),
            f["static_char_features"][bs].T.reshape(-1),
            f["bichar_left_features"][bs].T.reshape(-1),
            f["static_bichar_left_features"][bs].T.reshape(-1),
            f["bichar_right_features"][bs].T.reshape(-1),
            f["static_bichar_right_features"][bs].T.reshape(-1),
        ]
        for t in range(N_TILES):
            for j in range(6):
                idx_blk[:, t * 6 + j] = streams[j][t * 128:(t + 1) * 128]
        in_maps.append({"idx": idx_blk, **shared})
    return in_maps


_CACHED = {}


def kernel(**inputs):
    if "nc" not in _CACHED:
        _CACHED["nc"] = _build_program()
    nc = _CACHED["nc"]
    in_maps = _prep_host(inputs)
    res = run_bass_kernel_spmd(nc, in_maps, list(range(NCORES)))
    _CACHED["last_result"] = res
    out = np.empty((B_TOT, S, 2 * H), np.float32)
    for core in range(NCORES):
        # hs12 [2,S,128p,16=(2k,8b)] -> h[c,t,b,128k+p]; hs3 [2,S,44p,8b]
        h12 = res.results[core]["hs12"].astype(np.float32)
        h3 = res.results[core]["hs3"].astype(np.float32)
        hs = np.empty((2, S, BL, H), np.float32)
        hs[:, :, :, 0:256] = h12.reshape(2, S, 128, 2, 8).transpose(0, 1, 4, 3, 2).reshape(2, S, 8, 256)
        hs[:, :, :, 256:300] = h3.transpose(0, 1, 3, 2)
        bs = slice(core * BL, (core + 1) * BL)
        out[bs, :, 0:H] = hs[0].transpose(1, 0, 2)
        out[bs, :, H:2 * H] = hs[1, ::-1].transpose(1, 0, 2)
    return out


if __name__ == "__main__":
    sys.path.insert(0, os.path.dirname(os.path.abspath(__file__)))
    import reference
    inp = reference.setup_inputs()
    got = kernel(**{k: np.asarray(v) for k, v in inp.items()})
    exp = np.asarray(reference.reference(**inp))
    err = np.abs(got - exp)
    rel = err.max() / np.abs(exp).max()
    print("Relative error:", rel)



# revision 41
# speedup vs baseline: 1.1262x; 1.1137x over previous
"""Trainium2 Bass kernel for nn_Encoder_WordLstm (bi-LSTM over char/bichar embeddings).

Sharding: data-parallel over batch. Each of the 8 cores handles 8 sentences and
runs BOTH LSTM directions as two decoupled dependency chains that interleave on
the engines. Matmul operands are bf16 (fp32 PSUM accumulate); embedding tables
are pre-cast to bf16 (halves gather DMA traffic and makes PE transposes
1 cycle/row).

Per-core pipeline (all on device):
  1. indirect-DMA gathers: char/schar streams shared by both sides; bichar
     per side -> feat [128tok, 400+400] bf16 (all in forward token order)
  2. PE transpose -> featT, matmul W_lin + tanh -> linT [300, 128tok] bf16
  3. matmul Wih (bias via ones-row augmentation) -> x tiles (bf16) -> DRAM
  4. 512-step LSTM recurrence, both chains emitted op-interleaved:
     - gates = x + h @ WhhT via h-stationary matmuls (hT tiles as lhsT, Whh
       streamed); x and Whh-tail rows ride the k3 matmul via an I8-augmented
       stationary; the right chain reads x at reversed step indices.
     - ACT: sigmoid(i,f) fused, tanh(g), sigmoid(o); DVE: A=i*g~, D=f*c,
       c'=A+D.
     - transposed tail: PE-transpose o and c' into PSUM [128,24], tanh(c'T)
       and hT = oT*tanh(c'T) written straight into the double-buffered hT
       state tiles (no per-step copies); hs output DMA'd transposed.
Host reassembles/unpermutes to [64, 512, 600] f32.
"""

import os
import sys

import numpy as np

sys.path.insert(0, "/opt/trn_rl_repo")

import concourse.bass as bass
import concourse.bacc as bacc
import concourse.mybir as mybir
import concourse.tile as tile
from concourse.bass_utils import run_bass_kernel_spmd
from concourse.masks import make_identity

F32 = mybir.dt.float32
BF16 = mybir.dt.bfloat16
I32 = mybir.dt.int32
AF = mybir.ActivationFunctionType
ALU = mybir.AluOpType

B_TOT, S = 64, 512
DC = DB = 200
HID = H = 300
VC, VB = 10000, 200000
NCORES = 8
BL = B_TOT // NCORES          # 8 sentences per core
T = BL * S                    # 4096 tokens per core
G4 = 4 * H                    # 1200

# smoke-test overrides (break numerics, only to exercise compile/run quickly)
N_TILES = int(os.environ.get("K_NTILES", T // 128))   # 32
STEPS = int(os.environ.get("K_STEPS", S))             # 512
ABL = os.environ.get("K_ABL", "")          # ablation flags: hs,xdma,tp

# gate permutation: torch/ref order (i,f,g,o) -> kernel order (i,f,o,g)
_PERM = np.r_[0:300, 300:600, 900:1200, 600:900]

M300 = [128, 128, 44]         # chunks of 300 (lin output dims / recurrence h)
N512 = [(0, 512), (512, 512), (1024, 176)]  # free-dim chunks of 1200
KXP = [128, 128, 65]          # xproj contraction chunks (65 = 44 dims + ones@64)
RB = 32                       # right chain's partition base (32-aligned)


def _build_program():
    nc = bacc.Bacc()

    idx_d = nc.declare_dram_parameter("idx", [128, N_TILES * 6], I32, isOutput=False)
    tab_char = nc.declare_dram_parameter("char_embed", [VC, DC], BF16, isOutput=False)
    tab_schar = nc.declare_dram_parameter("static_char_embed", [VC, DC], BF16, isOutput=False)
    tab_bi = nc.declare_dram_parameter("bichar_embed", [VB, DB], BF16, isOutput=False)
    tab_sbi = nc.declare_dram_parameter("static_bichar_embed", [VB, DB], BF16, isOutput=False)
    wlin_d = nc.declare_dram_parameter("wlin_blk", [128, 24 * 128], BF16, isOutput=False)
    blin_d = nc.declare_dram_parameter("blin_blk", [128, 3], F32, isOutput=False)
    wih_d = nc.declare_dram_parameter("wihaug_blk", [128, 2 * 3 * G4], BF16, isOutput=False)
    whh12_d = nc.declare_dram_parameter("whh12_blk", [128, 2 * 2 * G4], BF16, isOutput=False)
    whh3_d = nc.declare_dram_parameter("whh3_blk", [44, 2 * G4], BF16, isOutput=False)
    i8_d = nc.declare_dram_parameter("i8blk", [8, 8], BF16, isOutput=False)
    ones_d = nc.declare_dram_parameter("onesblk", [1, 128], BF16, isOutput=False)
    hs12_d = nc.declare_dram_parameter("hs12", [2, S, 128, 16], BF16, isOutput=True)
    hs3_d = nc.declare_dram_parameter("hs3", [2, S, 44, 8], BF16, isOutput=True)
    x_d = nc.dram_tensor("x_seq", [2, T, G4], BF16)

    tables = [tab_char, tab_schar, tab_bi, tab_sbi]

    with tile.TileContext(nc) as tc:
        with (
            tc.tile_pool(name="const", bufs=1) as cp,
            tc.tile_pool(name="ph_sb", bufs=2) as pp,
            tc.tile_pool(name="rc_sb", bufs=2) as rp,
            tc.tile_pool(name="rc_h", bufs=4) as hp,
            tc.tile_pool(name="ps", bufs=1, space="PSUM") as psp,
        ):
            ident = cp.tile([128, 128], F32, tag="ident")
            make_identity(nc, ident[:, :])
            identb = cp.tile([128, 128], BF16, tag="identb")
            make_identity(nc, identb[:, :])
            idx_sb = cp.tile([128, N_TILES * 6], I32, tag="idx")
            nc.sync.dma_start(out=idx_sb[:, :], in_=idx_d[:, :])
            wlin_sb = cp.tile([128, 24 * 128], BF16, tag="wlin")
            nc.sync.dma_start(out=wlin_sb[:, :], in_=wlin_d[:, :])
            blin_sb = cp.tile([128, 3], F32, tag="blin")
            nc.sync.dma_start(out=blin_sb[:, :], in_=blin_d[:, :])
            wih_sb = cp.tile([128, 2 * 3 * G4], BF16, tag="wih")
            nc.sync.dma_start(out=wih_sb[:, :], in_=wih_d[:, :])
            whh12_sb = cp.tile([128, 2 * 2 * G4], BF16, tag="whh12")
            nc.sync.dma_start(out=whh12_sb[:, :], in_=whh12_d[:, :])

            # persistent linT tiles (side x parity); ones row 64 loaded once
            linTs = {}
            for side in range(2):
                for par in range(2):
                    lt = cp.tile([128, 3 * 128], BF16, tag=f"linT_{side}_{par}")
                    nc.sync.dma_start(out=lt[64:65, 256:384], in_=ones_d[:, :])
                    linTs[(side, par)] = lt

            # recurrence state, fully per-chain to keep the two scans decoupled.
            # hT state double-buffered (t%2) so the per-step hs-output DMA read
            # doesn't stall the next step's write (WAR slack of 2 steps).
            hT12s, hT3s, c_sts, b3s = [], [], [], []
            for c in range(2):
                p12, p3 = [], []
                for r in range(2):
                    t12 = cp.tile([128, 16], BF16, tag=f"hT12_{c}_{r}")
                    nc.vector.memset(t12[:, :], 0.0)
                    t3 = cp.tile([52, 8], BF16, tag=f"hT3_{c}_{r}")
                    nc.vector.memset(t3[0:44, :], 0.0)
                    nc.sync.dma_start(out=t3[44:52, 0:8], in_=i8_d[:, :])
                    p12.append(t12); p3.append(t3)
                cs = cp.tile([8, H], BF16, tag=f"c_{c}")
                nc.vector.memset(cs[:, :], 0.0)
                hT12s.append(p12); hT3s.append(p3); c_sts.append(cs)
                bufs = []
                for r in range(4):
                    b3 = cp.tile([52, G4], BF16, tag=f"b3_{c}_{r}")
                    nc.sync.dma_start(out=b3[0:44, :], in_=whh3_d[0:44, c * G4:(c + 1) * G4])
                    bufs.append(b3)
                b3s.append(bufs)

            # ---------------- phases 1-3: gather, transpose, linear, xproj ----
            def emit_tile(t):
                # char/schar gathers shared by both sides (right side consumes
                # x in reversed step order instead of gathering reversed)
                featcs = pp.tile([128, 400], BF16, tag="featcs")
                for j2 in range(2):
                    nc.gpsimd.indirect_dma_start(
                        out=featcs[:, 200 * j2:200 * (j2 + 1)],
                        out_offset=None,
                        in_=tables[j2][:, :],
                        in_offset=bass.IndirectOffsetOnAxis(
                            ap=idx_sb[:, t * 6 + j2:t * 6 + j2 + 1], axis=0),
                    )
                featTcs = pp.tile([128, 4 * 128], BF16, tag="ftcs")
                for kc in range(4):
                    tp = psp.tile([128, 128], BF16, tag="plg1", bufs=2)
                    nc.tensor.transpose(
                        tp[0:100, 0:128], featcs[:, kc * 100:(kc + 1) * 100],
                        identb[:, :])
                    nc.vector.tensor_copy(
                        featTcs[0:100, kc * 128:(kc + 1) * 128], tp[0:100, 0:128])
                for side in range(2):
                    featb = pp.tile([128, 400], BF16, tag=f"featb{side}")
                    for j2 in range(2):
                        col = t * 6 + 2 + side * 2 + j2
                        nc.gpsimd.indirect_dma_start(
                            out=featb[:, 200 * j2:200 * (j2 + 1)],
                            out_offset=None,
                            in_=tables[2 + j2][:, :],
                            in_offset=bass.IndirectOffsetOnAxis(
                                ap=idx_sb[:, col:col + 1], axis=0),
                        )
                    featTb = pp.tile([128, 4 * 128], BF16, tag=f"ftb{side}")
                    for kc in range(4):
                        tp = psp.tile([128, 128], BF16, tag="plg1", bufs=2)
                        nc.tensor.transpose(
                            tp[0:100, 0:128], featb[:, kc * 100:(kc + 1) * 100],
                            identb[:, :])
                        nc.vector.tensor_copy(
                            featTb[0:100, kc * 128:(kc + 1) * 128], tp[0:100, 0:128])
                    linT = linTs[(side, t % 2)]
                    for m in range(3):
                        mm = M300[m]
                        pl = psp.tile([128, 128], F32, tag="plg1", bufs=2)
                        for kc in range(8):
                            blk = (kc * 3 + m) * 128
                            rhsT = (featTcs if kc < 4 else featTb)
                            rkc = kc if kc < 4 else kc - 4
                            nc.tensor.matmul(
                                pl[0:mm, 0:128],
                                lhsT=wlin_sb[0:100, blk:blk + mm],
                                rhs=rhsT[0:100, rkc * 128:(rkc + 1) * 128],
                                start=(kc == 0), stop=(kc == 7))
                        nc.scalar.activation(
                            linT[0:mm, m * 128:m * 128 + 128],
                            pl[0:mm, 0:128], AF.Tanh,
                            bias=blin_sb[0:mm, m:m + 1])
                    px = psp.tile([128, G4], F32, tag="pxg0", bufs=2)
                    for kc in range(3):
                        kw = KXP[kc]
                        for (n0, nw) in N512:
                            nc.tensor.matmul(
                                px[:, n0:n0 + nw],
                                lhsT=linT[0:kw, kc * 128:kc * 128 + 128],
                                rhs=wih_sb[0:kw, (side * 3 + kc) * G4 + n0:
                                           (side * 3 + kc) * G4 + n0 + nw],
                                start=(kc == 0), stop=(kc == 2))
                    x_sb = pp.tile([128, G4], BF16, tag=f"x{side}")
                    for (n0, nw) in N512:
                        nc.scalar.copy(x_sb[:, n0:n0 + nw], px[:, n0:n0 + nw])
                    nc.sync.dma_start(
                        out=x_d[side, t * 128:(t + 1) * 128, :], in_=x_sb[:, :])

            # ---------------- phase 4: the two LSTM scans ---------------------
            # Per-chain dependency chains (independent, interleaved on engines).
            # c' = f*c + 2*(i*s) - i  where s = sigmoid(2g)  [tanh-free g path]
            # Op-level interleaving of the two chains: the ACT/DVE/Pool engine
            # queues are strict FIFO (exec-queue depth 0/8), so emitting chain
            # L's full step then chain R's causes head-of-line blocking (R's
            # ready sigmoid queued behind L's not-yet-ready tanh_c). Emitting
            # op-by-op across chains lets each engine alternate chains.
            abl = set(ABL.split(","))
            idn = identb[0:8, 0:8]
            idnf = ident[0:8, 0:8]

            def emit_step(t):
                ps_t, sg_t, so_t, tpx_t, tcT_t = {}, {}, {}, {}, {}
                rd, wr = (t + 1) % 2, t % 2
                for c in range(2):
                    b3 = b3s[c][t % 4]
                    tx = t if c == 0 else (S - 1 - t)
                    if "xdma" not in abl:
                        nc.sync.dma_start(
                            out=b3[44:52, :], in_=x_d[c, tx * 8:(tx + 1) * 8, :])
                    ps = psp.tile([8, G4], F32, tag="pxg0", bufs=2)
                    ps_t[c] = ps
                    for (n0, nw) in N512:
                        nc.tensor.matmul(
                            ps[:, n0:n0 + nw],
                            lhsT=hT12s[c][rd][:, 0:8],
                            rhs=whh12_sb[:, (c * 2) * G4 + n0:(c * 2) * G4 + n0 + nw],
                            start=True, stop=False)
                        nc.tensor.matmul(
                            ps[:, n0:n0 + nw],
                            lhsT=hT12s[c][rd][:, 8:16],
                            rhs=whh12_sb[:, (c * 2 + 1) * G4 + n0:
                                         (c * 2 + 1) * G4 + n0 + nw],
                            start=False, stop=False)
                        nc.tensor.matmul(
                            ps[:, n0:n0 + nw],
                            lhsT=hT3s[c][rd][0:52, 0:8],
                            rhs=b3[0:52, n0:n0 + nw], start=False, stop=True)
                # gate cols after PERM: i 0:300, f 300:600, o 600:900, g 900:1200
                for c in range(2):
                    sg = rp.tile([8, 600], BF16, tag=f"sg{c}")
                    sg_t[c] = sg
                    nc.scalar.activation(sg[:, 0:600], ps_t[c][:, 0:600], AF.Sigmoid)
                for c in range(2):
                    sgg = rp.tile([8, H], BF16, tag=f"sgg{c}")
                    nc.scalar.activation(sgg[:, :], ps_t[c][:, 900:1200], AF.Tanh)
                    sg_t[c] = (sg_t[c], sgg)
                for c in range(2):
                    so = rp.tile([8, H], BF16, tag=f"so{c}")
                    so_t[c] = so
                    nc.scalar.activation(so[:, :], ps_t[c][:, 600:900], AF.Sigmoid)
                Dts = {}
                for c in range(2):
                    Dt = rp.tile([8, H], BF16, tag=f"D{c}")
                    Dts[c] = Dt
                    nc.vector.tensor_tensor(
                        Dt[:, :], sg_t[c][0][:, 300:600], c_sts[c][:, :],
                        op=ALU.mult)
                Aps = {}
                for c in range(2):
                    Ap = rp.tile([8, H], BF16, tag=f"A{c}")
                    Aps[c] = Ap
                    nc.vector.tensor_tensor(
                        Ap[:, :], sg_t[c][0][:, 0:300], sg_t[c][1][:, :],
                        op=ALU.mult)
                for c in range(2):
                    nc.vector.tensor_tensor(
                        c_sts[c][:, :], Aps[c][:, :], Dts[c][:, :], op=ALU.add)
                # transposed tail: transpose o and c', tanh + h-mult in
                # [128,24] space straight into the hT state tiles (no copies)
                for c in range(2):
                    tpx = psp.tile([128, 48], BF16, tag="plg1", bufs=2)
                    tpx_t[c] = tpx
                    nc.tensor.transpose(tpx[:, 0:8], so_t[c][:, 0:128], idn)
                    nc.tensor.transpose(tpx[:, 8:16], so_t[c][:, 128:256], idn)
                    nc.tensor.transpose(tpx[0:44, 16:24], so_t[c][:, 256:300], idn)
                for c in range(2):
                    tpx = tpx_t[c]
                    nc.tensor.transpose(tpx[:, 24:32], c_sts[c][:, 0:128], idn)
                    nc.tensor.transpose(tpx[:, 32:40], c_sts[c][:, 128:256], idn)
                    nc.tensor.transpose(tpx[0:44, 40:48], c_sts[c][:, 256:300], idn)
                for c in range(2):
                    tcT = rp.tile([128, 24], BF16, tag=f"tcT{c}")
                    tcT_t[c] = tcT
                    nc.scalar.activation(tcT[:, 0:24], tpx_t[c][:, 24:48], AF.Tanh)
                for c in range(2):
                    nc.vector.tensor_tensor(
                        hT12s[c][wr][:, 0:16], tpx_t[c][:, 0:16],
                        tcT_t[c][:, 0:16], op=ALU.mult)
                    nc.vector.tensor_tensor(
                        hT3s[c][wr][0:44, 0:8], tpx_t[c][0:44, 16:24],
                        tcT_t[c][0:44, 16:24], op=ALU.mult)
                if "hs" not in abl:
                    for c in range(2):
                        nc.sync.dma_start(
                            out=hs12_d[c, t, :, :], in_=hT12s[c][wr][:, 0:16])
                        nc.sync.dma_start(
                            out=hs3_d[c, t, :, :], in_=hT3s[c][wr][0:44, 0:8])

            for k in range(N_TILES):
                emit_tile(k)
            for t in range(STEPS):
                emit_step(t)
    nc.compile()
    return nc


def _prep_host(inputs):
    """Build the per-core in_maps (host-side weight/index preprocessing)."""
    f = {k: np.asarray(v) for k, v in inputs.items()}

    wlinT = f["W_lin"].astype(np.float32).T            # [800, 300]
    wlin_blk = np.zeros((128, 24 * 128), np.float32)
    for kc in range(8):
        for m in range(3):
            mm = M300[m]
            blk = (kc * 3 + m) * 128
            wlin_blk[0:100, blk:blk + mm] = wlinT[kc * 100:(kc + 1) * 100,
                                                 m * 128:m * 128 + mm]
    blin_blk = np.zeros((128, 3), np.float32)
    for m in range(3):
        mm = M300[m]
        blin_blk[0:mm, m] = f["b_lin"][m * 128:m * 128 + mm]

    wih_blk = np.zeros((128, 2 * 3 * G4), np.float32)
    whh12_blk = np.zeros((128, 2 * 2 * G4), np.float32)
    whh3_blk = np.zeros((44, 2 * G4), np.float32)
    gscale = np.ones((G4,), np.float32)   # direct tanh on g: no prescale
    for c, sfx in enumerate(("l", "r")):
        wihT = (f[f"Wih_{sfx}"][_PERM, :].astype(np.float32) * gscale[:, None]).T
        bb = f[f"b_{sfx}"][_PERM].astype(np.float32) * gscale
        for kc in range(2):
            wih_blk[0:128, (c * 3 + kc) * G4:(c * 3 + kc + 1) * G4] = \
                wihT[kc * 128:(kc + 1) * 128, :]
        wih_blk[0:44, (c * 3 + 2) * G4:(c * 3 + 3) * G4] = wihT[256:300, :]
        wih_blk[64, (c * 3 + 2) * G4:(c * 3 + 3) * G4] = bb
        whhT = (f[f"Whh_{sfx}"][_PERM, :].astype(np.float32) * gscale[:, None]).T
        whh12_blk[:, (c * 2) * G4:(c * 2 + 1) * G4] = whhT[0:128, :]
        whh12_blk[:, (c * 2 + 1) * G4:(c * 2 + 2) * G4] = whhT[128:256, :]
        whh3_blk[:, c * G4:(c + 1) * G4] = whhT[256:300, :]

    import ml_dtypes
    bf = lambda a: a.astype(ml_dtypes.bfloat16)
    shared = {
        "char_embed": bf(f["char_embed"]),
        "static_char_embed": bf(f["static_char_embed"]),
        "bichar_embed": bf(f["bichar_embed"]),
        "static_bichar_embed": bf(f["static_bichar_embed"]),
        "wlin_blk": bf(wlin_blk), "blin_blk": blin_blk,
        "wihaug_blk": bf(wih_blk), "whh12_blk": bf(whh12_blk),
        "whh3_blk": bf(whh3_blk),
        "i8blk": bf(np.eye(8, dtype=np.float32)),
        "onesblk": bf(np.ones((1, 128), np.float32)),
    }

    in_maps = []
    for core in range(NCORES):
        bs = slice(core * BL, (core + 1) * BL)
        idx_blk = np.zeros((128, N_TILES * 6), np.int32)
        # stream order: [char schar bl sbl br sbr], all forward token order
        streams = [
            f["char_features"][bs].T.reshape(-1),
            f["static_char_features"][bs].T.reshape(-1),
            f["bichar_left_features"][bs].T.reshape(-1),
            f["static_bichar_left_features"][bs].T.reshape(-1),
            f["bichar_right_features"][bs].T.reshape(-1),
            f["static_bichar_right_features"][bs].T.reshape(-1),
        ]
        for t in range(N_TILES):
            for j in range(6):
                idx_blk[:, t * 6 + j] = streams[j][t * 128:(t + 1) * 128]
        in_maps.append({"idx": idx_blk, **shared})
    return in_maps


_CACHED = {}


def kernel(**inputs):
    if "nc" not in _CACHED:
        _CACHED["nc"] = _build_program()
    nc = _CACHED["nc"]
    in_maps = _prep_host(inputs)
    res = run_bass_kernel_spmd(nc, in_maps, list(range(NCORES)))
    _CACHED["last_result"] = res
    out = np.empty((B_TOT, S, 2 * H), np.float32)
    for core in range(NCORES):
        # hs12 [2,S,128p,16=(2k,8b)] -> h[c,t,b,128k+p]; hs3 [2,S,44p,8b]
        h12 = res.results[core]["hs12"].astype(np.float32)
        h3 = res.results[core]["hs3"].astype(np.float32)
        hs = np.empty((2, S, BL, H), np.float32)
        hs[:, :, :, 0:256] = h12.reshape(2, S, 128, 2, 8).transpose(0, 1, 4, 3, 2).reshape(2, S, 8, 256)
        hs[:, :, :, 256:300] = h3.transpose(0, 1, 3, 2)
        bs = slice(core * BL, (core + 1) * BL)
        out[bs, :, 0:H] = hs[0].transpose(1, 0, 2)
        out[bs, :, H:2 * H] = hs[1, ::-1].transpose(1, 0, 2)
    return out


if __name__ == "__main__":
    sys.path.insert(0, os.path.dirname(os.path.abspath(__file__)))
    import reference
    inp = reference.setup_inputs()
    got = kernel(**{k: np.asarray(v) for k, v in inp.items()})
    exp = np.asarray(reference.reference(**inp))
    err = np.abs(got - exp)
    rel = err.max() / np.abs(exp).max()
    print("Relative error:", rel)

